# revision 9
# baseline (speedup 1.0000x reference)
"""GQA (grouped-query attention) Trainium2 kernel, tensor-parallel across 8 NeuronCores.

Sharding: core c owns query heads [4c..4c+4) and kv head c (HQ=32, HK=8 -> the
4 query heads of a group share exactly the core's kv head). After attention the
per-core head outputs (attT, [512, S] fp16) are AllGathered, and each core then
computes a 512-column slice of the output projection, so no 32MB AllReduce is
needed -- the host just concatenates the 8 column slices.

All matmul inputs are fp16 (PE runs fp16 at full rate; PSUM accumulates fp32).
The host pre-transposes x and the weights so every contraction has its
reduction dim on the SBUF partition axis.
"""

import math
import os
import sys

import numpy as np

sys.path.insert(0, "/opt/trn_rl_repo")

import concourse.bacc as bacc  # noqa: E402
import concourse.bass as bass  # noqa: E402
import concourse.mybir as mybir  # noqa: E402
import concourse.tile as tile  # noqa: E402
from concourse.bass_utils import run_bass_kernel_spmd  # noqa: E402
from concourse.masks import make_identity  # noqa: E402

S = 2048
E = 4096
HQ = 32
HK = 8
D = 128
NCORES = 8
HQL = HQ // NCORES          # query heads per core
JQ = HQL * D                # 512 q-projection cols per core
P = 128
EK = E // P                 # 32 contraction chunks
SP = S // 512               # 4 s-passes of 512
SC = S // P                 # 16 seq chunks of 128
F16 = mybir.dt.float16
F32 = mybir.dt.float32
SCALE = 1.0 / math.sqrt(D)
NEG = -1e9


def build_nc_a():
    """Program A: QKV projection + RoPE + attention -> attT [512, S] fp16."""
    nc = bacc.Bacc("TRN2", target_bir_lowering=False, debug=False,
                   num_devices=NCORES)
    xt_d = nc.dram_tensor("xt", (E, S), F16, kind="ExternalInput")
    wqt_d = nc.dram_tensor("wqt", (E, JQ), F16, kind="ExternalInput")
    wkt_d = nc.dram_tensor("wkt", (E, D), F16, kind="ExternalInput")
    wvt_d = nc.dram_tensor("wvt", (E, D), F16, kind="ExternalInput")
    cos_d = nc.dram_tensor("cost", (D, S), F16, kind="ExternalInput")
    sin_d = nc.dram_tensor("sint", (D, S), F16, kind="ExternalInput")
    msk_d = nc.dram_tensor("maskneg", (P, P), F32, kind="ExternalInput")
    rt_d = nc.dram_tensor("rt", (P, P), F16, kind="ExternalInput")
    att_d = nc.dram_tensor("atto", (HQL * D, S), F16, kind="ExternalOutput")
    with tile.TileContext(nc) as tc:
        kernel_body(tc, xt_d, wqt_d, wkt_d, wvt_d, cos_d, sin_d,
                    msk_d, rt_d, att_d)
    nc.compile()
    return nc


def build_nc_b():
    """Program B: out[:, eslice] = att @ w_o[eslice, :].T (full j contraction)."""
    nc = bacc.Bacc("TRN2", target_bir_lowering=False, debug=False,
                   num_devices=NCORES)
    attf_d = nc.dram_tensor("attf", (HQ * D, S), F16, kind="ExternalInput")
    wot_d = nc.dram_tensor("wot", (HQ * D, 512), F16, kind="ExternalInput")
    out_d = nc.dram_tensor("out", (S, 512), F32, kind="ExternalOutput")
    with tile.TileContext(nc) as tc:
        with (
            tc.tile_pool(name="wpool", bufs=1) as wpool,
            tc.tile_pool(name="apool", bufs=4) as apool,
            tc.tile_pool(name="opool", bufs=3) as opool,
            tc.tile_pool(name="wops", bufs=8, space="PSUM") as wops,
        ):
            nc_ = tc.nc
            wo_sb = wpool.tile([P, EK * 512], F16)
            for k in range(EK):
                nc_.sync.dma_start(wo_sb[:, k * 512:(k + 1) * 512],
                                   wot_d[k * P:(k + 1) * P, :])
            for half in range(2):
                c0 = half * 1024
                ops = [wops.tile([P, 512], F32, tag="wo", name=f"wo{half}_{s8}")
                       for s8 in range(8)]
                for k in range(EK):
                    att_sb = apool.tile([P, 1024], F16, tag="att",
                                        name=f"att{half}_{k}")
                    nc_.sync.dma_start(att_sb[:],
                                       attf_d[k * P:(k + 1) * P, c0:c0 + 1024])
                    for s8 in range(8):
                        nc_.tensor.matmul(ops[s8][:],
                                          att_sb[:, s8 * P:(s8 + 1) * P],
                                          wo_sb[:, k * 512:(k + 1) * 512],
                                          start=(k == 0), stop=(k == EK - 1))
                for s8 in range(8):
                    o_sb = opool.tile([P, 512], F32, tag="o",
                                      name=f"o{half}_{s8}")
                    nc_.any.tensor_copy(o_sb[:], ops[s8][:])
                    sc = half * 8 + s8
                    nc_.sync.dma_start(out_d[sc * P:(sc + 1) * P, :], o_sb[:])
    nc.compile()
    return nc


def kernel_body(tc, xt_d, wqt_d, wkt_d, wvt_d, cos_d, sin_d,
                msk_d, rt_d, att_d):
    nc = tc.nc
    from contextlib import ExitStack
    with ExitStack() as stack:
        wpool = stack.enter_context(tc.tile_pool(name="wpool", bufs=1))
        _body(tc, stack, wpool, xt_d, wqt_d, wkt_d, wvt_d, cos_d,
              sin_d, msk_d, rt_d, att_d)


def _body(tc, stack, wpool, xt_d, wqt_d, wkt_d, wvt_d, cos_d, sin_d,
          msk_d, rt_d, att_d):
    nc = tc.nc
    # ---- resident SBUF tensors -------------------------------------------
    wq_sb = wpool.tile([P, EK * JQ], F16)      # wqT k-chunk k at cols [JQ*k)
    wk_sb = wpool.tile([P, EK * D], F16)
    wv_sb = wpool.tile([P, EK * D], F16)
    cos_sb = wpool.tile([P, S], F16)
    sin_sb = wpool.tile([P, S], F16)
    mask_sb = wpool.tile([P, P], F32)
    ident_sb = wpool.tile([P, P], F16)
    rt_sb = wpool.tile([P, P], F16)
    qrope = wpool.tile([P, HQL * S], F16)      # head h at cols [S*h)
    krope = wpool.tile([P, S], F16)
    vT_sb = wpool.tile([P, S], F16)            # [d, l]
    v_sb = wpool.tile([P, SC * D], F16)        # l-chunk lc at cols [D*lc): [l%128, d]
    attT_sb = wpool.tile([P, HQL * S], F16)    # [d, s] per head

    make_identity(nc, ident_sb[:])
    nc.sync.dma_start(cos_sb[:], cos_d[:])
    nc.sync.dma_start(sin_sb[:], sin_d[:])
    nc.sync.dma_start(mask_sb[:], msk_d[:])
    nc.sync.dma_start(rt_sb[:], rt_d[:])
    for k in range(EK):
        nc.sync.dma_start(wq_sb[:, k * JQ:(k + 1) * JQ],
                          wqt_d[k * P:(k + 1) * P, :])
        nc.sync.dma_start(wk_sb[:, k * D:(k + 1) * D],
                          wkt_d[k * P:(k + 1) * P, :])
        nc.sync.dma_start(wv_sb[:, k * D:(k + 1) * D],
                          wvt_d[k * P:(k + 1) * P, :])

    # ---- phase 1: QKV projections + RoPE + v transpose -------------------
    with (
        tc.tile_pool(name="xpool", bufs=3) as xpool,
        tc.tile_pool(name="evpool", bufs=3) as evpool,
        tc.tile_pool(name="tmppool", bufs=3) as tmppool,
        tc.tile_pool(name="pps", bufs=1, space="PSUM") as pps,
    ):
        for sp in range(SP):
            s0 = sp * 512
            qps = [pps.tile([P, 512], F32, tag="acc", bufs=6, name=f"qps{sp}_{j}")
                   for j in range(HQL)]
            kps = pps.tile([P, 512], F32, tag="acc", bufs=6, name=f"kps{sp}")
            vps = pps.tile([P, 512], F32, tag="acc", bufs=6, name=f"vps{sp}")
            for k in range(EK):
                xt_sb = xpool.tile([P, 512], F16, tag="xt", name=f"xt{sp}_{k}")
                nc.sync.dma_start(xt_sb[:], xt_d[k * P:(k + 1) * P, s0:s0 + 512])
                st = (k == 0)
                sp_ = (k == EK - 1)
                for j in range(HQL):
                    nc.tensor.matmul(qps[j][:], wq_sb[:, k * JQ + j * D: k * JQ + (j + 1) * D],
                                     xt_sb[:], start=st, stop=sp_)
                nc.tensor.matmul(kps[:], wk_sb[:, k * D:(k + 1) * D], xt_sb[:],
                                 start=st, stop=sp_)
                nc.tensor.matmul(vps[:], wv_sb[:, k * D:(k + 1) * D], xt_sb[:],
                                 start=st, stop=sp_)
            # evict + RoPE
            cs = cos_sb[:, s0:s0 + 512]
            sn = sin_sb[:, s0:s0 + 512]
            for j in range(HQL):
                q_sb = evpool.tile([P, 512], F16, tag="ev", name=f"qev{sp}_{j}")
                nc.scalar.copy(q_sb[:], qps[j][:])
                rot_ps = pps.tile([P, 512], F32, tag="rot", bufs=2,
                                  name=f"rq{sp}_{j}")
                nc.tensor.matmul(rot_ps[:], rt_sb[:], q_sb[:], start=True,
                                 stop=True)
                dst = qrope[:, j * S + s0: j * S + s0 + 512]
                _rope(nc, tmppool, dst, q_sb, rot_ps, cs, sn, f"q{sp}_{j}")
            k_sb = evpool.tile([P, 512], F16, tag="ev", name=f"kev{sp}")
            nc.scalar.copy(k_sb[:], kps[:])
            rot_ps = pps.tile([P, 512], F32, tag="rot", bufs=2, name=f"rk{sp}")
            nc.tensor.matmul(rot_ps[:], rt_sb[:], k_sb[:], start=True, stop=True)
            _rope(nc, tmppool, krope[:, s0:s0 + 512], k_sb, rot_ps, cs, sn,
                  f"k{sp}")
            # v: evict to vT then transpose 128-blocks into v_sb
            nc.scalar.copy(vT_sb[:, s0:s0 + 512], vps[:])
            for t in range(4):
                lc = sp * 4 + t
                vtp = pps.tile([P, P], F32, tag="rot", bufs=2, name=f"vtp{lc}")
                nc.tensor.matmul(vtp[:], vT_sb[:, s0 + t * P: s0 + (t + 1) * P],
                                 ident_sb[:], start=True, stop=True)
                nc.any.tensor_copy(v_sb[:, lc * D:(lc + 1) * D], vtp[:])

    # ---- phase 2: attention ---------------------------------------------
    with (
        tc.tile_pool(name="ppool", bufs=2) as ppool,
        tc.tile_pool(name="ptpool", bufs=SC) as ptpool,
        tc.tile_pool(name="rpool", bufs=8) as rpool,
        tc.tile_pool(name="dpool", bufs=2) as dpool,
        tc.tile_pool(name="spsum", bufs=2, space="PSUM") as spsum,
        tc.tile_pool(name="ptpsum", bufs=4, space="PSUM") as ptpsum,
        tc.tile_pool(name="otpsum", bufs=2, space="PSUM") as otpsum,
    ):
        for h in range(HQL):
            for ig in range(4):
                pt_tiles = [ptpool.tile([P, 512], F16, tag="pt",
                                        name=f"pt{h}_{ig}_{ls}")
                            for ls in range(4 * ig + 4)]
                for icl in range(4):
                    ic = 4 * ig + icl
                    L = P * (ic + 1)
                    nb = (L + 511) // 512
                    p_sb = ppool.tile([P, 2048], F16, tag="p", name=f"p{h}_{ic}")
                    rparts = rpool.tile([P, 4], F32, tag="rp", name=f"rp{h}_{ic}")
                    q_sl = qrope[:, h * S + ic * P: h * S + (ic + 1) * P]
                    for b in range(nb):
                        w = min(512, L - 512 * b)
                        sps = spsum.tile([P, 512], F32, tag="s", name=f"s{h}_{ic}_{b}")
                        nc.tensor.matmul(sps[:, :w], q_sl,
                                         krope[:, 512 * b: 512 * b + w],
                                         start=True, stop=True)
                        if b == nb - 1:
                            nc.vector.tensor_add(sps[:, w - P:w], sps[:, w - P:w],
                                                 mask_sb[:])
                        nc.scalar.activation(p_sb[:, 512 * b: 512 * b + w],
                                             sps[:, :w],
                                             mybir.ActivationFunctionType.Exp,
                                             scale=SCALE,
                                             accum_out=rparts[:, b:b + 1])
                    r32 = rpool.tile([P, 1], F32, tag="r", name=f"r{h}_{ic}")
                    if nb > 1:
                        nc.vector.reduce_sum(r32[:], rparts[:, :nb],
                                             axis=mybir.AxisListType.X)
                    else:
                        nc.vector.tensor_copy(r32[:], rparts[:, :1])
                    recip = rpool.tile([P, 1], F32, tag="rc", name=f"rc{h}_{ic}")
                    nc.vector.reciprocal(recip[:], r32[:])
                    diag = dpool.tile([P, P], F16, tag="dg", name=f"dg{h}_{ic}")
                    nc.vector.tensor_scalar_mul(diag[:], ident_sb[:], recip[:])
                    # transpose+normalize each 128-block of P: PT = P.T @ diag
                    for ls in range(ic + 1):
                        ptp = ptpsum.tile([P, P], F32, tag="ptp",
                                          name=f"ptp{h}_{ic}_{ls}")
                        nc.tensor.matmul(ptp[:], p_sb[:, ls * P:(ls + 1) * P],
                                         diag[:], start=True, stop=True)
                        nc.any.tensor_copy(pt_tiles[ls][:, icl * P:(icl + 1) * P],
                                           ptp[:])
                # PV for the whole 512-wide i-group
                otp = otpsum.tile([P, 512], F32, tag="ot", name=f"ot{h}_{ig}")
                nls = 4 * ig + 4
                for ls in range(nls):
                    cst = max(0, ls - 4 * ig) * P
                    nc.tensor.matmul(otp[:, cst:512],
                                     v_sb[:, ls * D:(ls + 1) * D],
                                     pt_tiles[ls][:, cst:512],
                                     start=(ls == 0), stop=(ls == nls - 1))
                nc.scalar.copy(attT_sb[:, h * S + ig * 512: h * S + (ig + 1) * 512],
                               otp[:])

    # ---- phase 3: write attention outputs ---------------------------------
    for h in range(HQL):
        nc.sync.dma_start(att_d[h * P:(h + 1) * P, :],
                          attT_sb[:, h * S:(h + 1) * S])


def _rope(nc, tmppool, dst, src, rot_ps, cs, sn, uid):
    """dst = src*cos + rot*sin; rot comes from the PE (signed permutation)."""
    tmp = tmppool.tile([P, 512], F16, tag="ropetmp", name=f"rt{uid}")
    nc.vector.tensor_mul(dst, src, cs)
    nc.vector.tensor_mul(tmp[:], rot_ps[:], sn)
    nc.vector.tensor_add(dst, dst, tmp[:])


# ---------------------------------------------------------------------------
# host side
# ---------------------------------------------------------------------------

_CACHE = {}


def _host_tables():
    pos = np.arange(S, dtype=np.float32)
    inv = 1.0 / (10000.0 ** (np.arange(0, D, 2, dtype=np.float32) / D))
    theta = pos[:, None] * inv[None, :]                  # [S, D/2]
    theta = np.concatenate([theta, theta], axis=-1)      # [S, D]
    cos = np.cos(theta).astype(np.float16)
    sin = np.sin(theta).astype(np.float16)
    cosT = np.ascontiguousarray(cos.T)                   # [D, S]
    sinT = np.ascontiguousarray(sin.T)
    mask = np.where(np.arange(P)[None, :] <= np.arange(P)[:, None],
                    0.0, NEG).astype(np.float32)         # [i, l]: 0 if l<=i
    rt = np.zeros((P, P), dtype=np.float16)              # rot = rt.T @ q
    for p in range(64):
        rt[p, p + 64] = 1.0                              # rot[d>=64] = q[d-64]
        rt[p + 64, p] = -1.0                             # rot[d<64] = -q[d+64]
    return cosT, sinT, mask, rt


def kernel(x, w_q, w_k, w_v, w_o):
    if "nca" not in _CACHE:
        _CACHE["nca"] = build_nc_a()
        _CACHE["ncb"] = build_nc_b()
    nca, ncb = _CACHE["nca"], _CACHE["ncb"]

    xt = np.ascontiguousarray(x.T).astype(np.float16)
    cosT, sinT, mask, rt = _host_tables()
    in_maps = []
    for c in range(NCORES):
        in_maps.append({
            "xt": xt,
            "wqt": np.ascontiguousarray(w_q[c * JQ:(c + 1) * JQ, :].T).astype(np.float16),
            "wkt": np.ascontiguousarray(w_k[c * D:(c + 1) * D, :].T).astype(np.float16),
            "wvt": np.ascontiguousarray(w_v[c * D:(c + 1) * D, :].T).astype(np.float16),
            "cost": cosT, "sint": sinT, "maskneg": mask, "rt": rt,
        })
    import time as _t
    _t0 = _t.time()
    res_a = run_bass_kernel_spmd(nca, in_maps, list(range(NCORES)))
    _CACHE["wall_a"] = _t.time() - _t0
    att_full = np.concatenate([res_a.results[c]["atto"] for c in range(NCORES)],
                              axis=0)                     # [HQ*D, S] fp16
    in_maps_b = []
    for c in range(NCORES):
        in_maps_b.append({
            "attf": att_full,
            "wot": np.ascontiguousarray(w_o[c * 512:(c + 1) * 512, :].T).astype(np.float16),
        })
    _t0 = _t.time()
    res_b = run_bass_kernel_spmd(ncb, in_maps_b, list(range(NCORES)))
    _CACHE["wall_b"] = _t.time() - _t0
    out = np.empty((S, E), dtype=np.float32)
    for c in range(NCORES):
        out[:, c * 512:(c + 1) * 512] = res_b.results[c]["out"]
    return out
